# revision 19
# baseline (speedup 1.0000x reference)
"""Trainium2 Bass kernel for nn_CalculateSLayer (GNN message passing).

Computes, for adj (N, N, 2) f32 and s (N, D) f32:
    a     = adj.sum(axis=2)                  # (N, N)
    s_in  = a.T @ s                          # (N, D)
    s_out = a @ s                            # (N, D)
returns (s_in, s_out) — matching the reference's output tuple.

Distribution: adjacency is sharded row-wise across 8 NeuronCores.  Core c
owns rows I_c = [c*512, (c+1)*512).  From its (512, 4096, 2) block it
computes on-device:
  * a partial s_in^T (D, N)    = (s[I_c]).T @ a[I_c]       (contracts i)
  * its exact  s_out^T (D,512) from a[I_c].T               (contracts j)
The host sums the 8 s_in partials and concatenates the s_out blocks.

Per-core dataflow (pipelined under Tile/Bacc; ~47 us HBM roofline):
  DMA : s_own/s_all are HOST-pre-shuffled into [128, tiles, 70] partition-
        major bf16 so each loads as ONE contiguous descriptor per partition
        (natural-layout 280 B descriptors cost ~20 ns of SDMA-engine time
        each and starve the adjacency stream ~6 us, on ANY queue).  The
        16.8 MB adjacency block streams as per-(chunk, i-tile) 512 KB loads
        with 4 KB/partition descriptors at ~390-420 GB/s; the first two
        chunks' loads are split across BOTH HWDGE rings (sync + scalar
        issue ~0.6 us per DMA instruction, so dual-ring issue doubles early
        queue depth and shortens the ramp).  The j-axis is chunked
        7 x 512 + 384 + 128: the final 128-column chunk is the only work
        gated on the last-arriving bytes, so the post-stream tail is ~8 PE
        ops instead of ~1.5 chunks of piled-up compute (DMA completions
        bunch at stream end because the slowest SDMA engine runs ~5-8%
        behind).  No SWDGE anywhere (its descriptor-ring fetches contend
        with SDMA engines 7/15).
  DVE : channel-reduce a_ch[i, j] = raw[i, j, 0] + raw[i, j, 1], casting to
        bf16 on write.
  PE  : all-bf16 (walrus rejects mixed 32/16-bit matmul inputs; bf16 runs
        LDWEIGHTS/moving at full rate, ~2x f32r transposes), f32 PSUM:
          s_in matmul  psum_sin(70,w) += s_own[it].T @ a_ch
          transposes   psT[t][j, it*128+i] = a_ch[i, t*128+j]  (bf16 ident)
          s_out matmul psum_out(70,512) += s_all[jt].T @ aT[t]
        each chunk's s_out matmuls run one chunk behind its transposes so
        the PE never stalls on PSUM evacuation.
  ACT/DVE : psT -> aT evacuations split across both engines (each copy has
        ~350 ns fixed overhead); s_in^T staged to bf16 (host sums partials
        in float64 — rounding ~1e-3 vs the 2e-2 gate) and flushed per
        chunk; final output DMAs issue from different engines (~0.9 us per
        DMA_DIRECT2D issue).

Fixed costs (measured with a 3-instruction probe): ~7 us framework
preamble + ~8.5 us exit teardown, independent of kernel content — the
optimization target is purely the DMA window + tail between them.
Measured rel L2 error vs the f32 reference: ~2.8e-3.
"""

import ml_dtypes
import numpy as np

import concourse.bass as bass
from concourse import bacc
import concourse.mybir as mybir
import concourse.tile as tile
from concourse import bass_utils

N = 4096          # nodes
D = 70            # embedding dim
NCORES = 8
RB = N // NCORES  # 512 rows per core
P = 128           # partitions
IT = RB // P      # 4 i-tiles per core
WJ = 512          # max j-chunk width
NJT = N // P      # 32 s_all subtiles

# j-chunking: 7 full chunks, then 384 + 128 so the last-arriving data has a
# minimal compute chain
CHUNKS = [(jc * WJ, WJ) for jc in range(7)] + [(3584, 384), (3968, 128)]
NCH = len(CHUNKS)

F32 = mybir.dt.float32
BF16 = mybir.dt.bfloat16

# Set by the test harness to capture a profile; the grading path leaves these
# untouched.
TRACE = False
TRACE_KWARGS = {}
LAST_RESULT = None


def _emit(nc: bass.Bass, adj_blk, s_own, s_all, s_inT, s_outT):
    with tile.TileContext(nc) as tc:
        with (
            # one buffer per (chunk, i-tile): no slot reuse, maximal prefetch
            tc.tile_pool(name="raw", bufs=NCH * IT) as raw_pool,
            tc.tile_pool(name="work", bufs=1) as work,
            tc.tile_pool(name="singles", bufs=1) as singles,
            tc.tile_pool(name="psT", bufs=1, space="PSUM") as psT_pool,
            tc.tile_pool(name="psSin", bufs=1, space="PSUM") as psSin_pool,
            tc.tile_pool(name="psOut", bufs=1, space="PSUM") as psOut_pool,
        ):
            # (i_tile, partition) view of the raw block
            adj_r = adj_blk.rearrange("(t p) j k -> p t j k", p=P)

            # constants + host-pre-shuffled s tensors on the ACT HWDGE ring:
            # contiguous per-partition lines, no tiny descriptors
            ident_dram = nc.inline_tensor(
                np.eye(P).astype(ml_dtypes.bfloat16), name="ident_const"
            )
            ident = singles.tile([P, P], BF16)
            nc.scalar.dma_start(out=ident, in_=ident_dram.ap())
            s_own_sb = singles.tile([P, IT, D], BF16)
            nc.scalar.dma_start(out=s_own_sb, in_=s_own)
            s_all_sb = singles.tile([P, NJT, D], BF16)
            nc.scalar.dma_start(out=s_all_sb, in_=s_all)

            # issue every raw load up front, per (chunk, i-tile).  The first
            # two chunks split their loads across both HWDGE rings so the
            # SDMA queues reach full depth ~2x sooner.
            raws = [[None] * IT for _ in range(NCH)]
            for k, (off, w) in enumerate(CHUNKS):
                for it in range(IT):
                    r = raw_pool.tile([P, w, 2], F32, tag="raw")
                    eng = nc.scalar if (k < 2 and it >= 2) else nc.sync
                    eng.dma_start(out=r, in_=adj_r[:, it, off : off + w, :])
                    raws[k][it] = r

            # persistent working tiles (bf16 transpose pipeline)
            a_chs = [
                [
                    work.tile([P, WJ], BF16, name=f"a_ch_{par}_{it}")
                    for it in range(IT)
                ]
                for par in range(2)
            ]
            aTs = [
                [work.tile([P, RB], BF16, name=f"aT_{par}_{t}") for t in range(4)]
                for par in range(2)
            ]
            sin_sb = [
                work.tile([D, w], BF16, name=f"sin_sb_{k}")
                for k, (_, w) in enumerate(CHUNKS)
            ]
            psT = [psT_pool.tile([P, RB], BF16, name=f"psT_{t}") for t in range(4)]
            psum_sins = [
                psSin_pool.tile([D, WJ], F32, name=f"psum_sin_{par}")
                for par in range(2)
            ]
            psum_out = psOut_pool.tile([D, RB], F32)

            def emit_sout_mm(k, t, stop=False):
                """One s_out^T accumulation for chunk k, subtile t (aT
                already evacuated; runs one chunk behind so the PE never
                stalls on the PSUM->SBUF copies)."""
                jt = CHUNKS[k][0] // P + t
                nc.tensor.matmul(
                    psum_out,
                    lhsT=s_all_sb[:, jt, :],
                    rhs=aTs[k % 2][t],
                    start=(jt == 0),
                    stop=stop,
                )

            for k, (off, w) in enumerate(CHUNKS):
                par = k % 2
                jt_n = w // P
                psum_sin = psum_sins[par]
                for it in range(IT):
                    raw = raws[k][it]
                    a_ch = a_chs[par][it]
                    nc.vector.tensor_add(
                        out=a_ch[:, :w], in0=raw[:, :, 0], in1=raw[:, :, 1]
                    )
                    # a^T tiles: psT[t][j, it*128 + i] = a[i, off + t*128 + j]
                    for t in range(jt_n):
                        nc.tensor.transpose(
                            psT[t][:, it * P : (it + 1) * P],
                            a_ch[:, t * P : (t + 1) * P],
                            ident,
                        )
                    # s_in^T partial: psum_sin[d, j] += sum_i s_own[i, d]*a[i, j]
                    nc.tensor.matmul(
                        psum_sin[:, :w],
                        lhsT=s_own_sb[:, it, :],
                        rhs=a_ch[:, :w],
                        start=(it == 0),
                        stop=(it == IT - 1),
                    )
                    if it == IT - 1 and k > 0:
                        # previous chunk's s_out matmuls: their aT operands
                        # finished copying while this chunk transposed
                        for t in range(CHUNKS[k - 1][1] // P):
                            emit_sout_mm(k - 1, t)
                # evacuate s_in^T chunk (DVE, f32->bf16) and a^T tiles
                # (alternating ACT/DVE — ~350 ns fixed cost per copy), then
                # flush s_in^T (ACT ring)
                nc.vector.tensor_copy(out=sin_sb[k], in_=psum_sin[:, :w])
                if k < NCH - 1:
                    for t in range(jt_n):
                        if t % 2 == 0:
                            nc.scalar.copy(out=aTs[par][t], in_=psT[t])
                        else:
                            nc.vector.tensor_copy(out=aTs[par][t], in_=psT[t])
                    nc.scalar.dma_start(out=s_inT[k], in_=sin_sb[k])
                else:
                    # final 128-wide chunk: one psT tile, evacuated in two
                    # halves (ACT lo / DVE hi) right behind its transposes
                    H = RB // 2
                    nc.scalar.copy(out=aTs[par][0][:, :H], in_=psT[0][:, :H])
                    nc.vector.tensor_copy(
                        out=aTs[par][0][:, H:], in_=psT[0][:, H:]
                    )
                    nc.scalar.dma_start(out=s_inT[k], in_=sin_sb[k])
                    emit_sout_mm(k, 0, stop=True)
            # s_out^T: evacuate lo on ACT, hi on DVE, flush as one DMA from
            # the idle SP engine
            s_outT_sb = singles.tile([D, RB], F32)
            nc.scalar.copy(out=s_outT_sb[:, : RB // 2], in_=psum_out[:, : RB // 2])
            nc.vector.tensor_copy(
                out=s_outT_sb[:, RB // 2 :], in_=psum_out[:, RB // 2 :]
            )
            nc.sync.dma_start(out=s_outT, in_=s_outT_sb)


def _build() -> bass.Bass:
    nc = bacc.Bacc("TRN2", num_devices=NCORES)
    adj_blk = nc.dram_tensor("adj_blk", [RB, N, 2], F32, kind="ExternalInput")
    # host-pre-shuffled tile layouts: i = t*128 + p  /  j = t*128 + p
    # (bf16: walrus rejects mixed 32/16-bit matmul inputs, so the whole
    # PE pipeline runs bf16 with f32 PSUM accumulation)
    s_own = nc.dram_tensor("s_own", [P, IT, D], BF16, kind="ExternalInput")
    s_all = nc.dram_tensor("s_all", [P, NJT, D], BF16, kind="ExternalInput")
    # one output tensor per flush so the output DMAs carry no cross-queue
    # write-ordering waits (HWDGE descriptors allow a single sync wait)
    s_inT = [
        nc.dram_tensor(f"s_inT_{k}", [D, w], BF16, kind="ExternalOutput")
        for k, (_, w) in enumerate(CHUNKS)
    ]
    s_outT = nc.dram_tensor("s_outT", [D, RB], F32, kind="ExternalOutput")
    _emit(
        nc,
        adj_blk.ap(),
        s_own.ap(),
        s_all.ap(),
        [t.ap() for t in s_inT],
        s_outT.ap(),
    )
    nc.finalize()
    return nc


_nc_cache = None


def kernel(adj: np.ndarray, s: np.ndarray):
    global _nc_cache, LAST_RESULT
    adj = np.ascontiguousarray(np.asarray(adj, dtype=np.float32))
    s = np.ascontiguousarray(np.asarray(s, dtype=np.float32))
    assert adj.shape == (N, N, 2) and s.shape == (N, D)

    if _nc_cache is None:
        _nc_cache = _build()
    nc = _nc_cache

    # partition-major tile shuffles so every DMA line is contiguous
    s_all_h = np.ascontiguousarray(
        s.reshape(NJT, P, D).transpose(1, 0, 2)
    ).astype(ml_dtypes.bfloat16)  # [p, jt, d], j = jt*128 + p
    in_maps = [
        {
            "adj_blk": np.ascontiguousarray(adj[c * RB : (c + 1) * RB]),
            "s_own": np.ascontiguousarray(
                s[c * RB : (c + 1) * RB].reshape(IT, P, D).transpose(1, 0, 2)
            ).astype(ml_dtypes.bfloat16),
            "s_all": s_all_h,
        }
        for c in range(NCORES)
    ]
    res = bass_utils.run_bass_kernel_spmd(
        nc,
        in_maps,
        core_ids=list(range(NCORES)),
        trace=TRACE,
        **TRACE_KWARGS,
    )
    LAST_RESULT = res

    s_in = (
        np.sum(
            [
                np.concatenate(
                    [r[f"s_inT_{k}"].astype(np.float64) for k in range(NCH)],
                    axis=1,
                )
                for r in res.results
            ],
            axis=0,
        )
        .astype(np.float32)
        .T
    )
    s_out = np.concatenate([r["s_outT"].T for r in res.results], axis=0)
    return (np.ascontiguousarray(s_in), np.ascontiguousarray(s_out))
